# revision 1
# baseline (speedup 1.0000x reference)
"""Single-head causal attention kernel for Trainium2 (Bass/Tile), SPMD over 8 cores.

Problem: inputs [B=8, S=2048, E=1024]; Wq/Wk/Wv [E, H=1024]; bq/bk/bv [H].
  q = x@Wq+bq; k = x@Wk+bk; v = x@Wv+bv
  out = softmax(causal(q k^T / sqrt(H))) v        -> [B, S, H]

Sharding: data-parallel over batch, 1 batch element per NeuronCore (8 cores).

Per-core dataflow (all matmuls fp32r = full-rate fp32 path):
  phase A: stream x, PE-transpose to xT [e,s]; K^T[h,s] = Wk^T x^T (resident)
  phase B: Q^T[h,s] -> DRAM scratch (SBUF can't hold Q^T+K^T+V at once)
  phase C: re-stream+transpose x; V[s,h] (resident; bias via rank-1 matmul)
  phase 2: per q-chunk (256 cols): scores^T[k,q] matmuls (causal tiles skipped),
           exp(x/32) fused on ScalarE, edge mask via gpsimd.affine_select,
           Z = ones-matmul column sums, O[q,h] = attnT^T V with 1/Z fused into
           the PSUM eviction (vector.tensor_scalar_mul).
"""

import numpy as np

import concourse.bass as bass
import concourse.bacc as bacc
import concourse.mybir as mybir
from concourse import tile
from concourse import bass_utils
from concourse.masks import make_identity

P = 128
F32 = mybir.dt.float32
F32R = mybir.dt.float32r

B, S, E, H = 8, 2048, 1024, 1024
QC = 256          # q-chunk width in attention phase
N_CORES = 8


def r(ap):
    """View an fp32 AP as float32r for full-rate TensorE matmuls."""
    return ap.bitcast(F32R)


def attention_kernel(tc, out, x, wq, bq, wk, bk, wv, bv, S=S, E=E, H=H, QC=QC):
    nc = tc.nc
    ST, ET, HT = S // P, E // P, H // P     # 128-tiles per dim
    NSC = S // 512                          # 512-wide s-chunks
    NQC = S // QC                           # q-chunks
    HCW = min(512, H)                       # h-chunk width
    HC = H // HCW
    inv_sqrt_h = 1.0 / float(np.sqrt(H))

    from contextlib import ExitStack

    root = ExitStack()
    with root:
        # ---- constants ----
        const = root.enter_context(tc.tile_pool(name="const", bufs=1))
        ident = const.tile([P, P], F32, name="ident")
        make_identity(nc, ident)
        ones_col = const.tile([P, 1], F32, name="ones_col")
        nc.gpsimd.memset(ones_col, 1.0)
        ones_row_f32 = const.tile([1, P], F32, name="ones_row_f32")
        nc.gpsimd.memset(ones_row_f32, 1.0)
        ones_row = const.tile([1, P], F32R, name="ones_row")
        nc.scalar.activation(ones_row[:], ones_row_f32[:],
                             mybir.ActivationFunctionType.Identity)
        bk_sb = const.tile([P, HT], F32, name="bk_sb")
        nc.sync.dma_start(bk_sb[:], bk.rearrange("(t p) -> p t", p=P))
        bq_sb = const.tile([P, HT], F32, name="bq_sb")
        nc.sync.dma_start(bq_sb[:], bq.rearrange("(t p) -> p t", p=P))
        bv_sb = const.tile([1, H], F32R, name="bv_sb")
        nc.sync.dma_start(bv_sb[:], bv.rearrange("(o h) -> o h", o=1).bitcast(F32R))

        # ---- resident arrays (K^T spans phases A..2; V allocated at phase C) ----
        kt_pool = root.enter_context(tc.tile_pool(name="kt", bufs=1))
        kT = [kt_pool.tile([P, S], F32R, name=f"kT{t}") for t in range(HT)]

        # ---- DRAM scratch for Q^T ----
        dram = root.enter_context(tc.tile_pool(name="dram", bufs=1, space="DRAM"))
        qt_dram = dram.tile([P, HT, S], F32R, name="qt_dram")

        # ================= phases A+B: xT once, K^T resident, Q^T -> DRAM ======
        with ExitStack() as ph:
            xT_pool = ph.enter_context(tc.tile_pool(name="xT", bufs=1))
            xT = [xT_pool.tile([P, S], F32R, name=f"xT{t}") for t in range(ET)]

            with ExitStack() as pha:
                x_pool = pha.enter_context(tc.tile_pool(name="x_in", bufs=4))
                tps = pha.enter_context(
                    tc.tile_pool(name="tpsum", bufs=4, space="PSUM"))
                w_pool = pha.enter_context(tc.tile_pool(name="wk", bufs=1))
                wk_all = w_pool.tile([P, ET, H], F32R, name="wk_all")
                for e in range(ET):
                    nc.scalar.dma_start(
                        wk_all[:, e, :], wk[e * P:(e + 1) * P, :].bitcast(F32R))
                mpsum = pha.enter_context(
                    tc.tile_pool(name="mpsum", bufs=4, space="PSUM"))

                for c in range(NSC):            # 512-row s-chunks
                    for ss in range(4):         # 128-row s-tiles within chunk
                        i = 4 * c + ss
                        x_t = x_pool.tile([P, E], F32, name="x_t")
                        nc.sync.dma_start(x_t[:], x[i * P:(i + 1) * P, :])
                        for t in range(ET):
                            tp = tps.tile([P, P], F32, name="tp", space="PSUM")
                            nc.tensor.transpose(tp[:], x_t[:, t * P:(t + 1) * P],
                                                ident[:])
                            dst = xT[t][:, i * P:(i + 1) * P]
                            if (i * ET + t) % 2 == 0:
                                nc.scalar.activation(
                                    dst, tp[:],
                                    mybir.ActivationFunctionType.Identity)
                            else:
                                nc.vector.tensor_copy(dst, tp[:])
                    # K^T for this s-chunk
                    for t in range(HT):
                        kp = mpsum.tile([P, 512], F32, name="kp", space="PSUM")
                        for e in range(ET):
                            nc.tensor.matmul(
                                kp[:],
                                wk_all[:, e, t * P:(t + 1) * P],
                                xT[e][:, c * 512:(c + 1) * 512],
                                start=(e == 0), stop=(e == ET - 1))
                        if t % 2 == 0:
                            nc.scalar.activation(
                                kT[t][:, c * 512:(c + 1) * 512], kp[:],
                                mybir.ActivationFunctionType.Identity,
                                bias=bk_sb[:, t:t + 1])
                        else:
                            nc.vector.tensor_scalar_add(
                                kT[t][:, c * 512:(c + 1) * 512], kp[:],
                                bk_sb[:, t:t + 1])

            # ---- phase B: Q^T -> DRAM ----
            with ExitStack() as phb:
                w_poolq = phb.enter_context(tc.tile_pool(name="wq", bufs=1))
                wq_all = w_poolq.tile([P, ET, H], F32R, name="wq_all")
                for e in range(ET):
                    nc.scalar.dma_start(
                        wq_all[:, e, :], wq[e * P:(e + 1) * P, :].bitcast(F32R))
                mpsum = phb.enter_context(
                    tc.tile_pool(name="mpsumq", bufs=6, space="PSUM"))
                qt_stage = phb.enter_context(tc.tile_pool(name="qt_stage", bufs=2))
                for c in range(NSC):
                    qs = qt_stage.tile([P, HT, 512], F32R, name="qs")
                    for t in range(HT):
                        qp = mpsum.tile([P, 512], F32, name="qp", space="PSUM")
                        for e in range(ET):
                            nc.tensor.matmul(
                                qp[:],
                                wq_all[:, e, t * P:(t + 1) * P],
                                xT[e][:, c * 512:(c + 1) * 512],
                                start=(e == 0), stop=(e == ET - 1))
                        if t % 2 == 0:
                            nc.scalar.activation(
                                qs[:, t, :], qp[:],
                                mybir.ActivationFunctionType.Identity,
                                bias=bq_sb[:, t:t + 1])
                        else:
                            nc.vector.tensor_scalar_add(
                                qs[:, t, :], qp[:], bq_sb[:, t:t + 1])
                    nc.sync.dma_start(
                        qt_dram[:, :, c * 512:(c + 1) * 512], qs[:])

        # ================= phase C: V resident (x re-streamed + re-transposed) ==
        ph_c2 = root.enter_context(ExitStack())
        v_pool = ph_c2.enter_context(tc.tile_pool(name="v", bufs=1))
        v_sb = [v_pool.tile([P, H], F32R, name=f"v{i}") for i in range(ST)]
        with ExitStack() as phc:
            w_poolv = phc.enter_context(tc.tile_pool(name="wv", bufs=1))
            wv_all = w_poolv.tile([P, ET, H], F32R, name="wv_all")
            for e in range(ET):
                nc.scalar.dma_start(
                    wv_all[:, e, :], wv[e * P:(e + 1) * P, :].bitcast(F32R))
            x_pool2 = phc.enter_context(tc.tile_pool(name="x_in2", bufs=2))
            xTc_pool = phc.enter_context(tc.tile_pool(name="xTc", bufs=2))
            tps2 = phc.enter_context(tc.tile_pool(name="tpsum2", bufs=4,
                                                  space="PSUM"))
            vpsum = phc.enter_context(tc.tile_pool(name="vpsum", bufs=3,
                                                   space="PSUM"))
            for i in range(ST):
                x_t = x_pool2.tile([P, E], F32, name="x_t2")
                nc.sync.dma_start(x_t[:], x[i * P:(i + 1) * P, :])
                xTc = xTc_pool.tile([P, ET, P], F32R, name="xTc")
                for t in range(ET):
                    tp = tps2.tile([P, P], F32, name="tp2", space="PSUM")
                    nc.tensor.transpose(tp[:], x_t[:, t * P:(t + 1) * P], ident[:])
                    if t % 2 == 0:
                        nc.scalar.activation(
                            xTc[:, t, :], tp[:],
                            mybir.ActivationFunctionType.Identity)
                    else:
                        nc.vector.tensor_copy(xTc[:, t, :], tp[:])
                for hc in range(HC):
                    vp = vpsum.tile([P, HCW], F32, name="vp", space="PSUM")
                    # bias row: V[s,h] starts at bv[h]
                    nc.tensor.matmul(vp[:], ones_row[:, :],
                                     bv_sb[:, hc * HCW:(hc + 1) * HCW],
                                     start=True, stop=False)
                    for e in range(ET):
                        nc.tensor.matmul(
                            vp[:],
                            xTc[:, e, :],
                            wv_all[:, e, hc * HCW:(hc + 1) * HCW],
                            start=False, stop=(e == ET - 1))
                    nc.vector.tensor_copy(v_sb[i][:, hc * HCW:(hc + 1) * HCW],
                                          vp[:])

        # ================= phase 2: attention ==================================
        with ExitStack() as ph2:
            qt_pool = ph2.enter_context(tc.tile_pool(name="qt_c", bufs=2))
            attn_pool = ph2.enter_context(
                tc.tile_pool(name="attnT", bufs=(S // P) + 2))
            o_pool = ph2.enter_context(tc.tile_pool(name="o_stage", bufs=3))
            rz_pool = ph2.enter_context(tc.tile_pool(name="rz", bufs=4))
            spsum = ph2.enter_context(tc.tile_pool(name="spsum", bufs=2,
                                                   space="PSUM"))
            zpsum = ph2.enter_context(tc.tile_pool(name="zpsum", bufs=2,
                                                   space="PSUM"))
            opsum = ph2.enter_context(tc.tile_pool(name="opsum", bufs=4,
                                                   space="PSUM"))
            QSUB = QC // P                       # q-subtiles per chunk
            for j in range(NQC):
                nk = ((j + 1) * QC) // P         # causal: k-tiles for this chunk
                qt_c = qt_pool.tile([P, HT, QC], F32R, name="qt_c")
                nc.sync.dma_start(qt_c[:], qt_dram[:, :, j * QC:(j + 1) * QC])
                attnT = []
                for i in range(nk):
                    sp = spsum.tile([P, QC], F32, name="sp", space="PSUM")
                    for t in range(HT):
                        nc.tensor.matmul(
                            sp[:],
                            kT[t][:, i * P:(i + 1) * P],
                            qt_c[:, t, :],
                            start=(t == 0), stop=(t == HT - 1))
                    at = attn_pool.tile([P, QC], F32R, name="at")
                    nc.scalar.activation(at[:], sp[:],
                                         mybir.ActivationFunctionType.Exp,
                                         scale=inv_sqrt_h)
                    if (i + 1) * P > j * QC:     # tile touches the diagonal
                        # keep where q >= k:  (j*QC - i*P) + f - p >= 0
                        nc.gpsimd.affine_select(
                            out=at[:], in_=at[:],
                            compare_op=mybir.AluOpType.is_ge,
                            fill=0.0,
                            base=j * QC - i * P,
                            channel_multiplier=-1,
                            pattern=[[1, QC]])
                    attnT.append(at)
                rz = rz_pool.tile([P, QSUB], F32, name="rz")
                for qs in range(QSUB):
                    zp = zpsum.tile([P, 1], F32, name="zp", space="PSUM")
                    for i in range(nk):
                        nc.tensor.matmul(
                            zp[:],
                            attnT[i][:, qs * P:(qs + 1) * P].bitcast(F32),
                            ones_col[:, :],
                            start=(i == 0), stop=(i == nk - 1))
                    nc.vector.reciprocal(rz[:, qs:qs + 1], zp[:])
                for qs in range(QSUB):
                    o_stage = o_pool.tile([P, H], F32, name="o_stage")
                    for hc in range(HC):
                        op = opsum.tile([P, HCW], F32, name="op", space="PSUM")
                        for i in range(nk):
                            nc.tensor.matmul(
                                op[:],
                                attnT[i][:, qs * P:(qs + 1) * P],
                                v_sb[i][:, hc * HCW:(hc + 1) * HCW],
                                start=(i == 0), stop=(i == nk - 1))
                        nc.vector.tensor_scalar_mul(
                            o_stage[:, hc * HCW:(hc + 1) * HCW], op[:],
                            rz[:, qs:qs + 1])
                    row = j * QC + qs * P
                    nc.sync.dma_start(out[row:row + P, :], o_stage[:])


def build_program(S=S, E=E, H=H, QC=QC, n_cores=N_CORES):
    nc = bacc.Bacc("TRN2", target_bir_lowering=False, debug=False,
                   num_devices=n_cores)
    x = nc.dram_tensor("x", [S, E], F32, kind="ExternalInput").ap()
    wq = nc.dram_tensor("wq", [E, H], F32, kind="ExternalInput").ap()
    bq = nc.dram_tensor("bq", [H], F32, kind="ExternalInput").ap()
    wk = nc.dram_tensor("wk", [E, H], F32, kind="ExternalInput").ap()
    bk = nc.dram_tensor("bk", [H], F32, kind="ExternalInput").ap()
    wv = nc.dram_tensor("wv", [E, H], F32, kind="ExternalInput").ap()
    bv = nc.dram_tensor("bv", [H], F32, kind="ExternalInput").ap()
    out = nc.dram_tensor("out", [S, H], F32, kind="ExternalOutput").ap()
    with tile.TileContext(nc) as tc:
        attention_kernel(tc, out, x, wq, bq, wk, bk, wv, bv,
                         S=S, E=E, H=H, QC=QC)
    nc.compile()
    return nc


def kernel(inputs, Wq, bq, Wk, bk, Wv, bv, _trace=False, _tmpdir=None):
    inputs = np.ascontiguousarray(inputs, dtype=np.float32)
    nc = build_program()
    in_maps = []
    for c in range(N_CORES):
        in_maps.append({
            "x": np.ascontiguousarray(inputs[c]),
            "wq": np.ascontiguousarray(Wq, dtype=np.float32),
            "bq": np.ascontiguousarray(bq, dtype=np.float32),
            "wk": np.ascontiguousarray(Wk, dtype=np.float32),
            "bk": np.ascontiguousarray(bk, dtype=np.float32),
            "wv": np.ascontiguousarray(Wv, dtype=np.float32),
            "bv": np.ascontiguousarray(bv, dtype=np.float32),
        })
    res = bass_utils.run_bass_kernel_spmd(
        nc, in_maps, core_ids=list(range(N_CORES)),
        trace=_trace, tmpdir=_tmpdir)
    out = np.stack([res.results[c]["out"] for c in range(N_CORES)], axis=0)
    if _trace:
        kernel.last_results = res
    return out



# revision 4
# speedup vs baseline: 1.2516x; 1.2516x over previous
"""Single-head causal attention kernel for Trainium2 (Bass/Tile), SPMD over 8 cores.

Problem: inputs [B=8, S=2048, E=1024]; Wq/Wk/Wv [E, H=1024]; bq/bk/bv [H].
  q = x@Wq+bq; k = x@Wk+bk; v = x@Wv+bv
  out = softmax(causal(q k^T / sqrt(H))) v        -> [B, S, H]

Sharding: data-parallel over batch, 1 batch element per NeuronCore (8 cores).

v2 dataflow (single pass over x, everything SBUF-resident, bf16 matmuls):
  - x streamed once, converted to bf16 (DVE/Pool), PE-transposed to xT [e,s]
    (bf16 transposes = 1 cyc/row); xT stays resident for the whole kernel.
  - weights DMA'd fp32, converted to bf16 once (ACT).
  - K^T[h,s] resident bf16 (bias fused into PSUM eviction); V[s,h] resident
    bf16 WITHOUT bias (bv folded into the final output: out = AV/Z + bv since
    softmax rows sum to 1).
  - attention per 256-col q-chunk, software-pipelined emission so PE never
    waits on evictions: qt(j) -> scores(j-? ) interleave:
        qt(0), S(0), [qt(j+1), Z(j), AV(j), S(j+1)] ..., Z(last), AV(last)
    qt computed just-in-time from resident xT (no DRAM scratch roundtrip).
  - AV eviction fuses 1/Z scale + bv add in one DVE scalar_tensor_tensor.
"""

import numpy as np

import concourse.bass as bass
import concourse.bacc as bacc
import concourse.mybir as mybir
from concourse import tile
from concourse import bass_utils
from concourse.masks import make_identity

P = 128
F32 = mybir.dt.float32
BF16 = mybir.dt.bfloat16

B, S, E, H = 8, 2048, 1024, 1024
QC = 256          # q-chunk width in attention phase
N_CORES = 8


def attention_kernel(tc, out, x, wq, bq, wk, bk, wv, bv, S=S, E=E, H=H, QC=QC):
    nc = tc.nc
    ST, ET, HT = S // P, E // P, H // P     # 16, 8, 8
    NSC = S // 512                          # 4 512-wide s-chunks
    NQC = S // QC                           # q-chunks
    QSUB = QC // P                          # q-subtiles per chunk
    HCW = 512
    HC = H // HCW
    inv_sqrt_h = 1.0 / float(np.sqrt(H))

    from contextlib import ExitStack

    root = ExitStack()
    with root:
        # ---- constants ----
        const = root.enter_context(tc.tile_pool(name="const", bufs=1))
        ident = const.tile([P, P], BF16, name="ident")
        make_identity(nc, ident)
        ones_col = const.tile([P, 1], BF16, name="ones_col")
        nc.gpsimd.memset(ones_col, 1.0)
        ones_row = const.tile([1, P], F32, name="ones_row")
        nc.gpsimd.memset(ones_row, 1.0)
        bk_sb = const.tile([P, HT], F32, name="bk_sb")
        nc.sync.dma_start(bk_sb[:], bk.rearrange("(t p) -> p t", p=P))
        bq_sb = const.tile([P, HT], F32, name="bq_sb")
        nc.sync.dma_start(bq_sb[:], bq.rearrange("(t p) -> p t", p=P))
        bv_sb = const.tile([1, H], F32, name="bv_sb")
        nc.sync.dma_start(bv_sb[:], bv.rearrange("(o h) -> o h", o=1))
        # bv broadcast to all partitions (for the fused output bias add)
        B_bv = const.tile([P, H], F32, name="B_bv")

        # ---- resident arrays ----
        xt_pool = root.enter_context(tc.tile_pool(name="xt", bufs=1))
        xT = [xt_pool.tile([P, S], BF16, name=f"xT{t}") for t in range(ET)]
        kt_pool = root.enter_context(tc.tile_pool(name="kt", bufs=1))
        kT = [kt_pool.tile([P, S], BF16, name=f"kT{t}") for t in range(HT)]
        v_pool = root.enter_context(tc.tile_pool(name="v", bufs=1))
        v_sb = [v_pool.tile([P, H], BF16, name=f"v{i}") for i in range(ST)]

        # ---- weights: DMA fp32, convert to bf16 on ACT ----
        w_pool = root.enter_context(tc.tile_pool(name="w", bufs=1))
        wk_sb = w_pool.tile([P, ET, H], BF16, name="wk_sb")
        wq_sb = w_pool.tile([P, ET, H], BF16, name="wq_sb")
        wv_sb = w_pool.tile([P, ET, H], BF16, name="wv_sb")

        # ================= phase A: xT + K^T, then V ===========================
        with ExitStack() as pha:
            wstage = pha.enter_context(tc.tile_pool(name="wstage", bufs=3))
            for wdram, wsb in ((wk, wk_sb), (wq, wq_sb), (wv, wv_sb)):
                for e in range(ET):
                    st = wstage.tile([P, H], F32, name="wst")
                    nc.scalar.dma_start(st[:], wdram[e * P:(e + 1) * P, :])
                    nc.scalar.activation(wsb[:, e, :], st[:],
                                         mybir.ActivationFunctionType.Identity)

            x_pool = pha.enter_context(tc.tile_pool(name="x_in", bufs=3))
            xb_pool = pha.enter_context(tc.tile_pool(name="xb", bufs=3))
            tps = pha.enter_context(tc.tile_pool(name="tpsum", bufs=4,
                                                 space="PSUM"))
            mpsum = pha.enter_context(tc.tile_pool(name="mpsum", bufs=3,
                                                   space="PSUM"))

            # B_bv = ones_row^T @ bv (K=1 fp32 matmul, one-time)
            for hc in range(HC):
                bp = mpsum.tile([P, 512], F32, name="mp", space="PSUM")
                nc.tensor.matmul(bp[:], ones_row[:, :],
                                 bv_sb[:, hc * HCW:(hc + 1) * HCW],
                                 start=True, stop=True)
                nc.vector.tensor_copy(B_bv[:, hc * HCW:(hc + 1) * HCW], bp[:])

            def emit_T(c):          # transpose 512-row s-chunk c into xT
                for ss in range(4):
                    i = 4 * c + ss
                    x_t = x_pool.tile([P, E], F32, name="x_t")
                    nc.sync.dma_start(x_t[:], x[i * P:(i + 1) * P, :])
                    xb = xb_pool.tile([P, E], BF16, name="xb")
                    if i % 2 == 0:
                        nc.vector.tensor_copy(xb[:], x_t[:])
                    else:
                        nc.gpsimd.tensor_copy(xb[:], x_t[:])
                    for t in range(ET):
                        tp = tps.tile([P, P], BF16, name="tp", space="PSUM")
                        nc.tensor.transpose(tp[:], xb[:, t * P:(t + 1) * P],
                                            ident[:])
                        dst = xT[t][:, i * P:(i + 1) * P]
                        if (i * ET + t) % 2 == 0:
                            nc.scalar.activation(
                                dst, tp[:],
                                mybir.ActivationFunctionType.Identity)
                        else:
                            nc.vector.tensor_copy(dst, tp[:])

            def emit_K(c):          # K^T for 512-wide s-chunk c
                for t in range(HT):
                    kp = mpsum.tile([P, 512], F32, name="mp", space="PSUM")
                    for e in range(ET):
                        nc.tensor.matmul(
                            kp[:],
                            wk_sb[:, e, t * P:(t + 1) * P],
                            xT[e][:, c * 512:(c + 1) * 512],
                            start=(e == 0), stop=(e == ET - 1))
                    if t % 2 == 0:
                        nc.scalar.activation(
                            kT[t][:, c * 512:(c + 1) * 512], kp[:],
                            mybir.ActivationFunctionType.Identity,
                            bias=bk_sb[:, t:t + 1])
                    else:
                        nc.vector.tensor_scalar_add(
                            kT[t][:, c * 512:(c + 1) * 512], kp[:],
                            bk_sb[:, t:t + 1])

            def emit_V(i):          # V rows i*P..(i+1)*P (no bias)
                for hc in range(HC):
                    vp = mpsum.tile([P, 512], F32, name="mp", space="PSUM")
                    for e in range(ET):
                        nc.tensor.matmul(
                            vp[:],
                            xT[e][:, i * P:(i + 1) * P],
                            wv_sb[:, e, hc * HCW:(hc + 1) * HCW],
                            start=(e == 0), stop=(e == ET - 1))
                    dst = v_sb[i][:, hc * HCW:(hc + 1) * HCW]
                    if (i + hc) % 2 == 0:
                        nc.scalar.activation(
                            dst, vp[:], mybir.ActivationFunctionType.Identity)
                    else:
                        nc.vector.tensor_copy(dst, vp[:])

            # software-pipelined emission: transposes stay a chunk ahead of K^T
            emit_T(0)
            emit_T(1)
            emit_K(0)
            emit_T(2)
            emit_K(1)
            emit_T(3)
            emit_K(2)
            emit_V(0)
            emit_V(1)
            emit_K(3)
            for i in range(2, ST):
                emit_V(i)

        # ================= phase 2: attention ==================================
        with ExitStack() as ph2:
            qt_pool = ph2.enter_context(tc.tile_pool(name="qt_c", bufs=2))
            attn_pool = ph2.enter_context(
                tc.tile_pool(name="attnT", bufs=(S // P) + 4))
            o_pool = ph2.enter_context(tc.tile_pool(name="o_stage", bufs=3))
            rz_pool = ph2.enter_context(tc.tile_pool(name="rz", bufs=3))
            qpsum = ph2.enter_context(tc.tile_pool(name="qpsum", bufs=2,
                                                   space="PSUM"))
            spsum = ph2.enter_context(tc.tile_pool(name="spsum", bufs=2,
                                                   space="PSUM"))
            zpsum = ph2.enter_context(tc.tile_pool(name="zpsum", bufs=2,
                                                   space="PSUM"))
            opsum = ph2.enter_context(tc.tile_pool(name="opsum", bufs=2,
                                                   space="PSUM"))

            def emit_qt(j):         # Q^T chunk j from resident xT, + bias
                qt = qt_pool.tile([P, HT, QC], BF16, name="qt")
                for t in range(HT):
                    qp = qpsum.tile([P, QC], F32, name="qp", space="PSUM")
                    for e in range(ET):
                        nc.tensor.matmul(
                            qp[:],
                            wq_sb[:, e, t * P:(t + 1) * P],
                            xT[e][:, j * QC:(j + 1) * QC],
                            start=(e == 0), stop=(e == ET - 1))
                    if t % 2 == 0:
                        nc.scalar.activation(
                            qt[:, t, :], qp[:],
                            mybir.ActivationFunctionType.Identity,
                            bias=bq_sb[:, t:t + 1])
                    else:
                        nc.vector.tensor_scalar_add(
                            qt[:, t, :], qp[:], bq_sb[:, t:t + 1])
                return qt

            def emit_scores(j, qt):
                nk = ((j + 1) * QC) // P
                ats = []
                for i in range(nk):
                    sp = spsum.tile([P, QC], F32, name="sp", space="PSUM")
                    for t in range(HT):
                        nc.tensor.matmul(
                            sp[:],
                            kT[t][:, i * P:(i + 1) * P],
                            qt[:, t, :],
                            start=(t == 0), stop=(t == HT - 1))
                    at = attn_pool.tile([P, QC], BF16, name="at")
                    nc.scalar.activation(at[:], sp[:],
                                         mybir.ActivationFunctionType.Exp,
                                         scale=inv_sqrt_h)
                    if (i + 1) * P > j * QC:     # tile touches the diagonal
                        nc.gpsimd.affine_select(
                            out=at[:], in_=at[:],
                            compare_op=mybir.AluOpType.is_ge,
                            fill=0.0,
                            base=j * QC - i * P,
                            channel_multiplier=-1,
                            pattern=[[1, QC]])
                    ats.append(at)
                return ats

            def emit_ZAV(j, ats):
                nk = len(ats)
                rz = rz_pool.tile([P, QSUB], F32, name="rz")
                for qs in range(QSUB):
                    zp = zpsum.tile([P, 1], F32, name="zp", space="PSUM")
                    for i in range(nk):
                        nc.tensor.matmul(
                            zp[:],
                            ats[i][:, qs * P:(qs + 1) * P],
                            ones_col[:, :],
                            start=(i == 0), stop=(i == nk - 1))
                    nc.vector.reciprocal(rz[:, qs:qs + 1], zp[:])
                for qs in range(QSUB):
                    o_stage = o_pool.tile([P, H], F32, name="o_stage")
                    for hc in range(HC):
                        op = opsum.tile([P, HCW], F32, name="op", space="PSUM")
                        for i in range(nk):
                            nc.tensor.matmul(
                                op[:],
                                ats[i][:, qs * P:(qs + 1) * P],
                                v_sb[i][:, hc * HCW:(hc + 1) * HCW],
                                start=(i == 0), stop=(i == nk - 1))
                        # out = psum * (1/Z) + bv   (one DVE op)
                        nc.vector.scalar_tensor_tensor(
                            out=o_stage[:, hc * HCW:(hc + 1) * HCW],
                            in0=op[:],
                            scalar=rz[:, qs:qs + 1],
                            in1=B_bv[:, hc * HCW:(hc + 1) * HCW],
                            op0=mybir.AluOpType.mult,
                            op1=mybir.AluOpType.add)
                    row = j * QC + qs * P
                    nc.sync.dma_start(out[row:row + P, :], o_stage[:])

            qt = emit_qt(0)
            ats_prev = emit_scores(0, qt)
            for j in range(1, NQC):
                qt = emit_qt(j)
                emit_ZAV(j - 1, ats_prev)
                ats_prev = emit_scores(j, qt)
            emit_ZAV(NQC - 1, ats_prev)


def build_program(S=S, E=E, H=H, QC=QC, n_cores=N_CORES):
    nc = bacc.Bacc("TRN2", target_bir_lowering=False, debug=False,
                   num_devices=n_cores)
    x = nc.dram_tensor("x", [S, E], F32, kind="ExternalInput").ap()
    wq = nc.dram_tensor("wq", [E, H], F32, kind="ExternalInput").ap()
    bq = nc.dram_tensor("bq", [H], F32, kind="ExternalInput").ap()
    wk = nc.dram_tensor("wk", [E, H], F32, kind="ExternalInput").ap()
    bk = nc.dram_tensor("bk", [H], F32, kind="ExternalInput").ap()
    wv = nc.dram_tensor("wv", [E, H], F32, kind="ExternalInput").ap()
    bv = nc.dram_tensor("bv", [H], F32, kind="ExternalInput").ap()
    out = nc.dram_tensor("out", [S, H], F32, kind="ExternalOutput").ap()
    with tile.TileContext(nc) as tc:
        attention_kernel(tc, out, x, wq, bq, wk, bk, wv, bv,
                         S=S, E=E, H=H, QC=QC)
    nc.compile()
    return nc


def kernel(inputs, Wq, bq, Wk, bk, Wv, bv, _trace=False, _tmpdir=None):
    inputs = np.ascontiguousarray(inputs, dtype=np.float32)
    nc = build_program()
    in_maps = []
    for c in range(N_CORES):
        in_maps.append({
            "x": np.ascontiguousarray(inputs[c]),
            "wq": np.ascontiguousarray(Wq, dtype=np.float32),
            "bq": np.ascontiguousarray(bq, dtype=np.float32),
            "wk": np.ascontiguousarray(Wk, dtype=np.float32),
            "bk": np.ascontiguousarray(bk, dtype=np.float32),
            "wv": np.ascontiguousarray(Wv, dtype=np.float32),
            "bv": np.ascontiguousarray(bv, dtype=np.float32),
        })
    res = bass_utils.run_bass_kernel_spmd(
        nc, in_maps, core_ids=list(range(N_CORES)),
        trace=_trace, tmpdir=_tmpdir)
    out = np.stack([res.results[c]["out"] for c in range(N_CORES)], axis=0)
    if _trace:
        kernel.last_results = res
    return out


# revision 9
# speedup vs baseline: 1.3912x; 1.1115x over previous
"""Single-head causal attention kernel for Trainium2 (Bass/Tile), SPMD over 8 cores.

Problem: inputs [B=8, S=2048, E=1024]; Wq/Wk/Wv [E, H=1024]; bq/bk/bv [H].
  q = x@Wq+bq; k = x@Wk+bk; v = x@Wv+bv
  out = softmax(causal(q k^T / sqrt(H))) v        -> [B, S, H]

Sharding: data-parallel over batch, 1 batch element per NeuronCore (8 cores).

v2 dataflow (single pass over x, everything SBUF-resident, bf16 matmuls):
  - x streamed once, converted to bf16 (DVE/Pool), PE-transposed to xT [e,s]
    (bf16 transposes = 1 cyc/row); xT stays resident for the whole kernel.
  - weights DMA'd fp32, converted to bf16 once (ACT).
  - K^T[h,s] resident bf16 (bias fused into PSUM eviction); V[s,h] resident
    bf16 WITHOUT bias (bv folded into the final output: out = AV/Z + bv since
    softmax rows sum to 1).
  - attention per 256-col q-chunk, software-pipelined emission so PE never
    waits on evictions: qt(j) -> scores(j-? ) interleave:
        qt(0), S(0), [qt(j+1), Z(j), AV(j), S(j+1)] ..., Z(last), AV(last)
    qt computed just-in-time from resident xT (no DRAM scratch roundtrip).
  - AV eviction fuses 1/Z scale + bv add in one DVE scalar_tensor_tensor.
"""

import numpy as np

import concourse.bass as bass
import concourse.bacc as bacc
import concourse.mybir as mybir
from concourse import tile
from concourse import bass_utils
from concourse.masks import make_identity

P = 128
F32 = mybir.dt.float32
BF16 = mybir.dt.bfloat16

B, S, E, H = 8, 2048, 1024, 1024
QC = 256          # q-chunk width in attention phase
N_CORES = 8


def attention_kernel(tc, out, x, wq, bq, wk, bk, wv, bv, S=S, E=E, H=H, QC=QC):
    nc = tc.nc
    ST, ET, HT = S // P, E // P, H // P     # 16, 8, 8
    NSC = S // 512                          # 4 512-wide s-chunks
    NQC = S // QC                           # q-chunks
    QSUB = QC // P                          # q-subtiles per chunk
    HCW = 512
    HC = H // HCW
    inv_sqrt_h = 1.0 / float(np.sqrt(H))

    from contextlib import ExitStack

    root = ExitStack()
    with root:
        # ---- constants ----
        const = root.enter_context(tc.tile_pool(name="const", bufs=1))
        ident = const.tile([P, P], F32, name="ident")
        make_identity(nc, ident)
        ones_col = const.tile([P, 1], BF16, name="ones_col")
        nc.gpsimd.memset(ones_col, 1.0)
        ones_row = const.tile([1, P], F32, name="ones_row")
        nc.gpsimd.memset(ones_row, 1.0)
        bk_sb = const.tile([P, HT], F32, name="bk_sb")
        nc.sync.dma_start(bk_sb[:], bk.rearrange("(t p) -> p t", p=P))
        bq_sb = const.tile([P, HT], F32, name="bq_sb")
        nc.sync.dma_start(bq_sb[:], bq.rearrange("(t p) -> p t", p=P))
        bv_sb = const.tile([1, H], F32, name="bv_sb")
        nc.sync.dma_start(bv_sb[:], bv.rearrange("(o h) -> o h", o=1))
        # bv broadcast to all partitions (for the fused output bias add)
        B_bv = const.tile([P, H], F32, name="B_bv")

        # ---- resident arrays ----
        xt_pool = root.enter_context(tc.tile_pool(name="xt", bufs=1))
        xT = [xt_pool.tile([P, S], BF16, name=f"xT{t}") for t in range(ET)]
        kt_pool = root.enter_context(tc.tile_pool(name="kt", bufs=1))
        kT = [kt_pool.tile([P, S], BF16, name=f"kT{t}") for t in range(HT)]
        v_pool = root.enter_context(tc.tile_pool(name="v", bufs=1))
        v_sb = [v_pool.tile([P, H], BF16, name=f"v{i}") for i in range(ST)]

        # ---- weights: DMA fp32, convert to bf16 on ACT ----
        w_pool = root.enter_context(tc.tile_pool(name="w", bufs=1))
        wk_sb = w_pool.tile([P, ET, H], BF16, name="wk_sb")
        wq_sb = w_pool.tile([P, ET, H], BF16, name="wq_sb")
        wv_sb = w_pool.tile([P, ET, H], BF16, name="wv_sb")

        # ================= phase A: xT + K^T, then V ===========================
        with ExitStack() as pha:
            wstage = pha.enter_context(tc.tile_pool(name="wstage", bufs=4))

            def emit_wconv(wdram, wsb, e):
                # stage one fp32 weight slice, convert to bf16 on DVE
                st = wstage.tile([P, H], F32, name="wst")
                nc.scalar.dma_start(st[:], wdram[e * P:(e + 1) * P, :])
                nc.vector.tensor_copy(wsb[:, e, :], st[:])

            for e in range(ET):                 # wk needed first (K^T chunk 0)
                emit_wconv(wk, wk_sb, e)

            x_pool = pha.enter_context(tc.tile_pool(name="x_in", bufs=4))
            tps = pha.enter_context(tc.tile_pool(name="tpsum", bufs=4,
                                                 space="PSUM"))
            mpsum = pha.enter_context(tc.tile_pool(name="mpsum", bufs=4,
                                                   space="PSUM"))

            # B_bv = ones_row^T @ bv (K=1 fp32 matmul, one-time)
            for hc in range(HC):
                bp = mpsum.tile([P, 512], F32, name="mp", space="PSUM")
                nc.tensor.matmul(bp[:], ones_row[:, :],
                                 bv_sb[:, hc * HCW:(hc + 1) * HCW],
                                 start=True, stop=True)
                nc.vector.tensor_copy(B_bv[:, hc * HCW:(hc + 1) * HCW], bp[:])

            def emit_T(c, evict_dve=False):
                # transpose 512-row s-chunk c into xT (fp32 transpose, the
                # PSUM eviction converts to bf16)
                for ss in range(4):
                    i = 4 * c + ss
                    x_t = x_pool.tile([P, E], F32, name="x_t")
                    nc.sync.dma_start(x_t[:], x[i * P:(i + 1) * P, :])
                    for t in range(ET):
                        tp = tps.tile([P, P], F32, name="tp", space="PSUM")
                        nc.tensor.transpose(tp[:], x_t[:, t * P:(t + 1) * P],
                                            ident[:])
                        dst = xT[t][:, i * P:(i + 1) * P]
                        if evict_dve and (i * ET + t) % 2 == 0:
                            nc.vector.tensor_copy(dst, tp[:])
                        else:
                            nc.scalar.activation(
                                dst, tp[:],
                                mybir.ActivationFunctionType.Identity)

            def emit_K(c):          # K^T for 512-wide s-chunk c
                for t in range(HT):
                    kp = mpsum.tile([P, 512], F32, name="mp", space="PSUM")
                    for e in range(ET):
                        nc.tensor.matmul(
                            kp[:],
                            wk_sb[:, e, t * P:(t + 1) * P],
                            xT[e][:, c * 512:(c + 1) * 512],
                            start=(e == 0), stop=(e == ET - 1))
                    if t % 2 == 0:
                        nc.scalar.activation(
                            kT[t][:, c * 512:(c + 1) * 512], kp[:],
                            mybir.ActivationFunctionType.Identity,
                            bias=bk_sb[:, t:t + 1])
                    else:
                        nc.vector.tensor_scalar_add(
                            kT[t][:, c * 512:(c + 1) * 512], kp[:],
                            bk_sb[:, t:t + 1])

            def emit_V(i):          # V rows i*P..(i+1)*P (no bias)
                for hc in range(HC):
                    vp = mpsum.tile([P, 512], F32, name="mp", space="PSUM")
                    for e in range(ET):
                        nc.tensor.matmul(
                            vp[:],
                            xT[e][:, i * P:(i + 1) * P],
                            wv_sb[:, e, hc * HCW:(hc + 1) * HCW],
                            start=(e == 0), stop=(e == ET - 1))
                    dst = v_sb[i][:, hc * HCW:(hc + 1) * HCW]
                    if (i + hc) % 2 == 0:
                        nc.scalar.activation(
                            dst, vp[:], mybir.ActivationFunctionType.Identity)
                    else:
                        nc.vector.tensor_copy(dst, vp[:])

            # software-pipelined emission: transposes stay a chunk ahead of
            # K^T; wq/wv stage+convert interleaved so DVE never blocks
            # transpose evictions
            emit_T(0)
            emit_T(1)
            emit_K(0)
            for e in range(ET):
                emit_wconv(wq, wq_sb, e)
            emit_T(2, evict_dve=True)
            emit_K(1)
            for e in range(ET):
                emit_wconv(wv, wv_sb, e)
            emit_T(3, evict_dve=True)
            emit_K(2)
            emit_V(0)
            emit_V(1)
            emit_K(3)
            for i in range(2, ST):
                emit_V(i)

        # ================= phase 2: attention ==================================
        with ExitStack() as ph2:
            qt_pool = ph2.enter_context(tc.tile_pool(name="qt_c", bufs=2))
            attn_pool = ph2.enter_context(
                tc.tile_pool(name="attnT", bufs=(S // P) + 4))
            o_pool = ph2.enter_context(tc.tile_pool(name="o_stage", bufs=3))
            rz_pool = ph2.enter_context(tc.tile_pool(name="rz", bufs=3))
            qpsum = ph2.enter_context(tc.tile_pool(name="qpsum", bufs=2,
                                                   space="PSUM"))
            spsum = ph2.enter_context(tc.tile_pool(name="spsum", bufs=2,
                                                   space="PSUM"))
            zpsum = ph2.enter_context(tc.tile_pool(name="zpsum", bufs=2,
                                                   space="PSUM"))
            opsum = ph2.enter_context(tc.tile_pool(name="opsum", bufs=2,
                                                   space="PSUM"))

            def emit_qt(j):         # Q^T chunk j from resident xT, + bias
                qt = qt_pool.tile([P, HT, QC], BF16, name="qt")
                for t in range(HT):
                    qp = qpsum.tile([P, QC], F32, name="qp", space="PSUM")
                    for e in range(ET):
                        nc.tensor.matmul(
                            qp[:],
                            wq_sb[:, e, t * P:(t + 1) * P],
                            xT[e][:, j * QC:(j + 1) * QC],
                            start=(e == 0), stop=(e == ET - 1))
                    if t % 2 == 0:
                        nc.scalar.activation(
                            qt[:, t, :], qp[:],
                            mybir.ActivationFunctionType.Identity,
                            bias=bq_sb[:, t:t + 1])
                    else:
                        nc.vector.tensor_scalar_add(
                            qt[:, t, :], qp[:], bq_sb[:, t:t + 1])
                return qt

            def emit_scores(j, qt):
                nk = ((j + 1) * QC) // P
                ats = []
                for i in range(nk):
                    sp = spsum.tile([P, QC], F32, name="sp", space="PSUM")
                    for t in range(HT):
                        nc.tensor.matmul(
                            sp[:],
                            kT[t][:, i * P:(i + 1) * P],
                            qt[:, t, :],
                            start=(t == 0), stop=(t == HT - 1))
                    at = attn_pool.tile([P, QC], BF16, name="at")
                    nc.scalar.activation(at[:], sp[:],
                                         mybir.ActivationFunctionType.Exp,
                                         scale=inv_sqrt_h)
                    if (i + 1) * P > j * QC:     # tile touches the diagonal
                        nc.gpsimd.affine_select(
                            out=at[:], in_=at[:],
                            compare_op=mybir.AluOpType.is_ge,
                            fill=0.0,
                            base=j * QC - i * P,
                            channel_multiplier=-1,
                            pattern=[[1, QC]])
                    ats.append(at)
                return ats

            def emit_ZAV(j, ats):
                nk = len(ats)
                rz = rz_pool.tile([P, QSUB], F32, name="rz")
                for qs in range(QSUB):
                    zp = zpsum.tile([P, 1], F32, name="zp", space="PSUM")
                    for i in range(nk):
                        nc.tensor.matmul(
                            zp[:],
                            ats[i][:, qs * P:(qs + 1) * P],
                            ones_col[:, :],
                            start=(i == 0), stop=(i == nk - 1))
                    nc.vector.reciprocal(rz[:, qs:qs + 1], zp[:])
                for qs in range(QSUB):
                    o_stage = o_pool.tile([P, H], F32, name="o_stage")
                    for hc in range(HC):
                        op = opsum.tile([P, HCW], F32, name="op", space="PSUM")
                        for i in range(nk):
                            nc.tensor.matmul(
                                op[:],
                                ats[i][:, qs * P:(qs + 1) * P],
                                v_sb[i][:, hc * HCW:(hc + 1) * HCW],
                                start=(i == 0), stop=(i == nk - 1))
                        # out = psum * (1/Z) + bv   (one DVE op)
                        nc.vector.scalar_tensor_tensor(
                            out=o_stage[:, hc * HCW:(hc + 1) * HCW],
                            in0=op[:],
                            scalar=rz[:, qs:qs + 1],
                            in1=B_bv[:, hc * HCW:(hc + 1) * HCW],
                            op0=mybir.AluOpType.mult,
                            op1=mybir.AluOpType.add)
                    row = j * QC + qs * P
                    nc.sync.dma_start(out[row:row + P, :], o_stage[:])

            qt = emit_qt(0)
            ats_prev = emit_scores(0, qt)
            for j in range(1, NQC):
                qt = emit_qt(j)
                emit_ZAV(j - 1, ats_prev)
                ats_prev = emit_scores(j, qt)
            emit_ZAV(NQC - 1, ats_prev)


def build_program(S=S, E=E, H=H, QC=QC, n_cores=N_CORES):
    nc = bacc.Bacc("TRN2", target_bir_lowering=False, debug=False,
                   num_devices=n_cores)
    x = nc.dram_tensor("x", [S, E], F32, kind="ExternalInput").ap()
    wq = nc.dram_tensor("wq", [E, H], F32, kind="ExternalInput").ap()
    bq = nc.dram_tensor("bq", [H], F32, kind="ExternalInput").ap()
    wk = nc.dram_tensor("wk", [E, H], F32, kind="ExternalInput").ap()
    bk = nc.dram_tensor("bk", [H], F32, kind="ExternalInput").ap()
    wv = nc.dram_tensor("wv", [E, H], F32, kind="ExternalInput").ap()
    bv = nc.dram_tensor("bv", [H], F32, kind="ExternalInput").ap()
    out = nc.dram_tensor("out", [S, H], F32, kind="ExternalOutput").ap()
    with tile.TileContext(nc) as tc:
        attention_kernel(tc, out, x, wq, bq, wk, bk, wv, bv,
                         S=S, E=E, H=H, QC=QC)
    nc.compile()
    return nc


def kernel(inputs, Wq, bq, Wk, bk, Wv, bv, _trace=False, _tmpdir=None):
    inputs = np.ascontiguousarray(inputs, dtype=np.float32)
    nc = build_program()
    in_maps = []
    for c in range(N_CORES):
        in_maps.append({
            "x": np.ascontiguousarray(inputs[c]),
            "wq": np.ascontiguousarray(Wq, dtype=np.float32),
            "bq": np.ascontiguousarray(bq, dtype=np.float32),
            "wk": np.ascontiguousarray(Wk, dtype=np.float32),
            "bk": np.ascontiguousarray(bk, dtype=np.float32),
            "wv": np.ascontiguousarray(Wv, dtype=np.float32),
            "bv": np.ascontiguousarray(bv, dtype=np.float32),
        })
    res = bass_utils.run_bass_kernel_spmd(
        nc, in_maps, core_ids=list(range(N_CORES)),
        trace=_trace, tmpdir=_tmpdir)
    out = np.stack([res.results[c]["out"] for c in range(N_CORES)], axis=0)
    if _trace:
        kernel.last_results = res
    return out
